# revision 7
# baseline (speedup 1.0000x reference)
"""VQ codebook nearest-codeword lookup + VQ-VAE loss, data-parallel over 8 trn2 cores.

Math: for each 128-dim slot f_n, reference computes
  dist[n,k] = |f_n|^2 - 2 f_n.w_k + |w_k|^2,  j_n = argmin_k dist,
  out = W[j] regrouped, loss_b = mean_g 1.25 * dist[n, j_n].
We compute q[n,k] = f.w - (|f|^2 + |w|^2)/2 = -dist/2 on the tensor engine
(one PSUM accumulation group: a rank-2 bias matmul + the 128-contraction
main matmul), so argmax_k q = argmin_k dist and min-dist = -2 max q.
Per-row argmax uses the DVE max8/max_index instructions; codewords are
fetched with an indirect (gather) DMA; the grouped loss mean is one
[128x32] matmul against a block-diagonal coefficient matrix.

All per-core constants are packed into a single [128, CW] DRAM blob so the
whole preload is one DMA (one semaphore — the ISA allows very few sync
waits per instruction).
"""

import sys

if "/opt/trn_rl_repo" not in sys.path:
    sys.path.insert(0, "/opt/trn_rl_repo")

import numpy as np

EMBED = 128
FLEN = 512
K = 2048
BATCH = 8192
N = BATCH * FLEN // EMBED          # 32768 flattened slots
NCORES = 8
NS = N // NCORES                   # 4096 slots per core
TILES = NS // 128                  # 32 tiles of 128 slots
GROUPS = FLEN // EMBED             # 4 slots per feature row

# const blob column layout
C_FT = 0                            # [128, NS]      f^T shard
C_WT = C_FT + NS                    # [128, K]       W^T
C_BL = C_WT + K                     # [2, NS]        bias lhsT rows (ones, -|f|^2/2)
C_BR = C_BL + NS                    # [2, K]         bias rhs rows (-|w|^2/2, ones)
C_G = C_BR + K                      # [128, 32]      loss-mean coefficients
CW = C_G + 32

_cached_nc = None


def _build():
    global _cached_nc
    if _cached_nc is not None:
        return _cached_nc

    from concourse import bass, mybir
    from concourse.tile import TileContext

    f32 = mybir.dt.float32
    u32 = mybir.dt.uint32

    nc = bass.Bass()
    const_d = nc.declare_dram_parameter("consts", [128, CW], f32, isOutput=False)
    W_d = nc.declare_dram_parameter("W", [K, EMBED], f32, isOutput=False)
    out_d = nc.declare_dram_parameter("out", [NS, EMBED], f32, isOutput=True)
    loss_d = nc.declare_dram_parameter("loss", [32, TILES], f32, isOutput=True)

    Copy = mybir.ActivationFunctionType.Copy

    with TileContext(nc) as tc:
        with (
            tc.tile_pool(name="const", bufs=1) as cpool,
            tc.tile_pool(name="psum", bufs=2, space="PSUM") as ppool,
            tc.tile_pool(name="qsb", bufs=3) as qpool,
            tc.tile_pool(name="small", bufs=4) as spool,
            tc.tile_pool(name="wj", bufs=4) as wjpool,
        ):
            blob = cpool.tile([128, CW], f32)
            nc.sync.dma_start(out=blob[:], in_=const_d[:])
            fT_sb = blob[:, C_FT : C_FT + NS]
            WT_sb = blob[:, C_WT : C_WT + K]
            biasL_sb = blob[0:2, C_BL : C_BL + NS]
            biasR_sb = blob[0:2, C_BR : C_BR + K]
            G_sb = blob[:, C_G : C_G + 32]
            qmax_all = cpool.tile([128, TILES], f32)

            for t in range(TILES):
                ts = slice(128 * t, 128 * t + 128)
                q_ps = ppool.tile([128, K], f32, tag="q")
                for c in range(4):
                    cs = slice(512 * c, 512 * c + 512)
                    nc.tensor.matmul(
                        out=q_ps[:, cs],
                        lhsT=biasL_sb[:, ts],
                        rhs=biasR_sb[:, cs],
                        start=True,
                        stop=False,
                    )
                    nc.tensor.matmul(
                        out=q_ps[:, cs],
                        lhsT=fT_sb[:, ts],
                        rhs=WT_sb[:, cs],
                        start=False,
                        stop=True,
                    )
                q_sb = qpool.tile([128, K], f32)
                nc.scalar.activation(out=q_sb[:], in_=q_ps[:], func=Copy)
                top8 = spool.tile([128, 8], f32, tag="top8")
                nc.vector.max(out=top8[:], in_=q_sb[:])
                idx8 = spool.tile([128, 8], u32, tag="idx8")
                nc.vector.max_index(out=idx8[:], in_max=top8[:], in_values=q_sb[:])
                nc.scalar.activation(
                    out=qmax_all[:, t : t + 1], in_=top8[:, 0:1], func=Copy
                )
                wj = wjpool.tile([128, EMBED], f32)
                nc.gpsimd.indirect_dma_start(
                    out=wj[:],
                    out_offset=None,
                    in_=W_d[:],
                    in_offset=bass.IndirectOffsetOnAxis(ap=idx8[:, 0:1], axis=0),
                )
                nc.sync.dma_start(out=out_d[ts, :], in_=wj[:])

            loss_ps = ppool.tile([32, TILES], f32, tag="q")
            nc.tensor.matmul(
                out=loss_ps[:], lhsT=G_sb[:], rhs=qmax_all[:], start=True, stop=True
            )
            loss_sb = spool.tile([32, TILES], f32, tag="losssb")
            nc.scalar.activation(out=loss_sb[:], in_=loss_ps[:], func=Copy)
            nc.sync.dma_start(out=loss_d[:], in_=loss_sb[:])

    # This walrus build allows only ONE sync-wait on compute-engine
    # instructions (matmul/activation/DVE ops). Tile emits up to ~3.
    # Hoist all but one wait into standalone EventSemaphore instructions
    # on the same engine, placed immediately before the instruction —
    # semantically identical (all waits must pass before the op issues).
    nsplit = 0
    for bb in nc.m.functions[0].blocks:
        newlist = []
        for ins in bb.instructions:
            si = ins.sync_info
            if si is not None and len(si.on_wait) > 1:
                waits = list(si.on_wait)
                for w in waits[:-1]:
                    nsplit += 1
                    newlist.append(
                        mybir.InstEventSemaphore(
                            name=f"hoistwait-{nsplit}-{ins.name}",
                            engine=ins.engine,
                            sync_info=mybir.SyncInfo(on_wait=[w], on_update=[]),
                        )
                    )
                ins.sync_info = mybir.SyncInfo(
                    on_wait=[waits[-1]], on_update=si.on_update
                )
            newlist.append(ins)
        bb.instructions = newlist

    _cached_nc = nc
    return nc


def _prep_inputs(feature, W):
    f = np.ascontiguousarray(feature, dtype=np.float32).reshape(N, EMBED)
    Wc = np.ascontiguousarray(W, dtype=np.float32)
    wsq = np.einsum("kd,kd->k", Wc, Wc, dtype=np.float32)
    in_maps = []
    for c in range(NCORES):
        fs = f[c * NS : (c + 1) * NS]
        fsq = np.einsum("nd,nd->n", fs, fs, dtype=np.float32)
        blob = np.zeros((128, CW), np.float32)
        blob[:, C_FT : C_FT + NS] = fs.T
        blob[:, C_WT : C_WT + K] = Wc.T
        blob[0, C_BL : C_BL + NS] = 1.0
        blob[1, C_BL : C_BL + NS] = -0.5 * fsq
        blob[0, C_BR : C_BR + K] = -0.5 * wsq
        blob[1, C_BR : C_BR + K] = 1.0
        g = np.zeros((128, 32), np.float32)
        g[np.arange(128), np.arange(128) // GROUPS] = -0.625
        blob[:, C_G : C_G + 32] = g
        in_maps.append({"consts": blob, "W": Wc})
    return in_maps


def _finish(results):
    out = np.concatenate([r["out"] for r in results], axis=0).reshape(BATCH, FLEN)
    loss = np.concatenate(
        [np.ascontiguousarray(r["loss"].T).reshape(-1) for r in results], axis=0
    )
    return loss, out


def kernel(feature, W, _trace=False, _trace_kwargs=None):
    from concourse.bass_utils import run_bass_kernel_spmd

    nc = _build()
    in_maps = _prep_inputs(feature, W)
    res = run_bass_kernel_spmd(
        nc,
        in_maps,
        list(range(NCORES)),
        trace=_trace,
        **(_trace_kwargs or {}),
    )
    loss, out = _finish(res.results)
    if _trace:
        return (loss, out), res
    return (loss, out)


# revision 10
# speedup vs baseline: 2.9251x; 2.9251x over previous
"""VQ codebook nearest-codeword lookup + VQ-VAE loss, data-parallel over 8 trn2 cores.

Math: for each 128-dim slot f_n, reference computes
  dist[n,k] = |f_n|^2 - 2 f_n.w_k + |w_k|^2,  j_n = argmin_k dist,
  out = W[j] regrouped, loss_b = mean_g 1.25 * dist[n, j_n].
We compute q[n,k] = f.w - (|f|^2 + |w|^2)/2 = -dist/2 on the tensor engine
(one PSUM accumulation group: a rank-2 bias matmul + the 128-contraction
main matmul), so argmax_k q = argmin_k dist and min-dist = -2 max q.
Per-row argmax uses the DVE max8/max_index instructions; codewords are
fetched with an indirect (gather) DMA; the grouped loss mean is one
[128x32] matmul against a block-diagonal coefficient matrix.

All per-core constants are packed into a single [128, CW] DRAM blob so the
whole preload is one DMA (one semaphore — the ISA allows very few sync
waits per instruction).
"""

import sys

if "/opt/trn_rl_repo" not in sys.path:
    sys.path.insert(0, "/opt/trn_rl_repo")

import numpy as np

EMBED = 128
FLEN = 512
K = 2048
BATCH = 8192
N = BATCH * FLEN // EMBED          # 32768 flattened slots
NCORES = 8
NS = N // NCORES                   # 4096 slots per core
TILES = NS // 128                  # 32 tiles of 128 slots
GROUPS = FLEN // EMBED             # 4 slots per feature row

# bf16 const blob column layout. The f32 matmul runs as 2 slow HW passes
# (~4 cyc/col total); instead we split operands into bf16 hi+lo on the host
# and compute q = fh.wh + fh.wl + fl.wh with 3 full-rate bf16 matmuls
# (the dropped lo.lo term is ~4e-5 rms, far below the smallest argmax gap).
# The rank-1 norm biases use 3-way bf16 splits (exact to ~2^-24) folded
# into one contraction-6 bias matmul.
C_FTH = 0                           # [128, NS]  f^T hi
C_FTL = C_FTH + NS                  # [128, NS]  f^T lo
C_WTH = C_FTL + NS                  # [128, K]   W^T hi
C_WTL = C_WTH + K                   # [128, K]   W^T lo
C_BL = C_WTL + K                    # [6, NS]    bias lhsT rows
C_BR = C_BL + NS                    # [6, K]     bias rhs rows
CW = C_BR + K

_cached_nc = None


def _split_bf16(x, terms):
    """Split float32 array into `terms` bf16 arrays summing to ~x."""
    import ml_dtypes

    out = []
    rem = x.astype(np.float32)
    for _ in range(terms):
        h = rem.astype(ml_dtypes.bfloat16)
        out.append(h)
        rem = rem - h.astype(np.float32)
    return out


def _build():
    global _cached_nc
    if _cached_nc is not None:
        return _cached_nc

    from concourse import bass, mybir
    from concourse.tile import TileContext

    f32 = mybir.dt.float32
    bf16 = mybir.dt.bfloat16
    u32 = mybir.dt.uint32

    nc = bass.Bass()
    const_d = nc.declare_dram_parameter("consts", [128, CW], bf16, isOutput=False)
    G_d = nc.declare_dram_parameter("G", [128, 32], f32, isOutput=False)
    W_d = nc.declare_dram_parameter("W", [K, EMBED], f32, isOutput=False)
    out_d = nc.declare_dram_parameter("out", [NS, EMBED], f32, isOutput=True)
    loss_d = nc.declare_dram_parameter("loss", [32, TILES], f32, isOutput=True)

    Copy = mybir.ActivationFunctionType.Copy

    with TileContext(nc) as tc:
        with (
            tc.tile_pool(name="const", bufs=1) as cpool,
            tc.tile_pool(name="psum", bufs=2, space="PSUM") as ppool,
            tc.tile_pool(name="qsb", bufs=3) as qpool,
            tc.tile_pool(name="small", bufs=4) as spool,
            tc.tile_pool(name="wj", bufs=4) as wjpool,
        ):
            blob = cpool.tile([128, CW], bf16)
            nc.sync.dma_start(out=blob[:], in_=const_d[:])
            G_sb = cpool.tile([128, 32], f32)
            nc.sync.dma_start(out=G_sb[:], in_=G_d[:])
            fTh = blob[:, C_FTH : C_FTH + NS]
            fTl = blob[:, C_FTL : C_FTL + NS]
            WTh = blob[:, C_WTH : C_WTH + K]
            WTl = blob[:, C_WTL : C_WTL + K]
            biasL_sb = blob[0:6, C_BL : C_BL + NS]
            biasR_sb = blob[0:6, C_BR : C_BR + K]
            qmax_all = cpool.tile([128, TILES], f32)

            for t in range(TILES):
                ts = slice(128 * t, 128 * t + 128)
                q_ps = ppool.tile([128, K], f32, tag="q")
                for c in range(4):
                    cs = slice(512 * c, 512 * c + 512)
                    nc.tensor.matmul(
                        out=q_ps[:, cs],
                        lhsT=biasL_sb[:, ts],
                        rhs=biasR_sb[:, cs],
                        start=True,
                        stop=False,
                    )
                    nc.tensor.matmul(
                        out=q_ps[:, cs],
                        lhsT=fTh[:, ts],
                        rhs=WTh[:, cs],
                        start=False,
                        stop=False,
                    )
                    nc.tensor.matmul(
                        out=q_ps[:, cs],
                        lhsT=fTh[:, ts],
                        rhs=WTl[:, cs],
                        start=False,
                        stop=False,
                    )
                    nc.tensor.matmul(
                        out=q_ps[:, cs],
                        lhsT=fTl[:, ts],
                        rhs=WTh[:, cs],
                        start=False,
                        stop=True,
                    )
                q_sb = qpool.tile([128, K], f32)
                nc.scalar.activation(out=q_sb[:], in_=q_ps[:], func=Copy)
                top8 = spool.tile([128, 8], f32, tag="top8")
                nc.vector.max(out=top8[:], in_=q_sb[:])
                idx8 = spool.tile([128, 8], u32, tag="idx8")
                nc.vector.max_index(out=idx8[:], in_max=top8[:], in_values=q_sb[:])
                nc.scalar.activation(
                    out=qmax_all[:, t : t + 1], in_=top8[:, 0:1], func=Copy
                )
                wj = wjpool.tile([128, EMBED], f32)
                nc.gpsimd.indirect_dma_start(
                    out=wj[:],
                    out_offset=None,
                    in_=W_d[:],
                    in_offset=bass.IndirectOffsetOnAxis(ap=idx8[:, 0:1], axis=0),
                )
                nc.sync.dma_start(out=out_d[ts, :], in_=wj[:])

            loss_ps = ppool.tile([32, TILES], f32, tag="q")
            nc.tensor.matmul(
                out=loss_ps[:], lhsT=G_sb[:], rhs=qmax_all[:], start=True, stop=True
            )
            loss_sb = spool.tile([32, TILES], f32, tag="losssb")
            nc.scalar.activation(out=loss_sb[:], in_=loss_ps[:], func=Copy)
            nc.sync.dma_start(out=loss_d[:], in_=loss_sb[:])

    # This walrus build allows only ONE sync-wait on compute-engine
    # instructions (matmul/activation/DVE ops). Tile emits up to ~3.
    # Hoist all but one wait into standalone EventSemaphore instructions
    # on the same engine, placed immediately before the instruction —
    # semantically identical (all waits must pass before the op issues).
    nsplit = 0
    for bb in nc.m.functions[0].blocks:
        newlist = []
        for ins in bb.instructions:
            si = ins.sync_info
            if si is not None and len(si.on_wait) > 1:
                waits = list(si.on_wait)
                for w in waits[:-1]:
                    nsplit += 1
                    newlist.append(
                        mybir.InstEventSemaphore(
                            name=f"hoistwait-{nsplit}-{ins.name}",
                            engine=ins.engine,
                            sync_info=mybir.SyncInfo(on_wait=[w], on_update=[]),
                        )
                    )
                ins.sync_info = mybir.SyncInfo(
                    on_wait=[waits[-1]], on_update=si.on_update
                )
            newlist.append(ins)
        bb.instructions = newlist

    _cached_nc = nc
    return nc


def _prep_inputs(feature, W):
    import ml_dtypes

    bf = ml_dtypes.bfloat16
    f = np.ascontiguousarray(feature, dtype=np.float32).reshape(N, EMBED)
    Wc = np.ascontiguousarray(W, dtype=np.float32)
    wsq = np.einsum("kd,kd->k", Wc, Wc, dtype=np.float32)
    WTh, WTl = _split_bf16(Wc.T, 2)
    bh, bm, bl = _split_bf16(-0.5 * wsq, 3)
    g = np.zeros((128, 32), np.float32)
    g[np.arange(128), np.arange(128) // GROUPS] = -0.625
    in_maps = []
    for c in range(NCORES):
        fs = f[c * NS : (c + 1) * NS]
        fsq = np.einsum("nd,nd->n", fs, fs, dtype=np.float32)
        fTh, fTl = _split_bf16(fs.T, 2)
        ah, am, al = _split_bf16(-0.5 * fsq, 3)
        blob = np.zeros((128, CW), bf)
        blob[:, C_FTH : C_FTH + NS] = fTh
        blob[:, C_FTL : C_FTL + NS] = fTl
        blob[:, C_WTH : C_WTH + K] = WTh
        blob[:, C_WTL : C_WTL + K] = WTl
        # bias rows: ones x (wsq splits), (fsq splits) x ones
        blob[0, C_BL : C_BL + NS] = bf(1.0)
        blob[1, C_BL : C_BL + NS] = bf(1.0)
        blob[2, C_BL : C_BL + NS] = bf(1.0)
        blob[3, C_BL : C_BL + NS] = ah
        blob[4, C_BL : C_BL + NS] = am
        blob[5, C_BL : C_BL + NS] = al
        blob[0, C_BR : C_BR + K] = bh
        blob[1, C_BR : C_BR + K] = bm
        blob[2, C_BR : C_BR + K] = bl
        blob[3, C_BR : C_BR + K] = bf(1.0)
        blob[4, C_BR : C_BR + K] = bf(1.0)
        blob[5, C_BR : C_BR + K] = bf(1.0)
        in_maps.append({"consts": blob, "G": g, "W": Wc})
    return in_maps


def _finish(results):
    out = np.concatenate([r["out"] for r in results], axis=0).reshape(BATCH, FLEN)
    loss = np.concatenate(
        [np.ascontiguousarray(r["loss"].T).reshape(-1) for r in results], axis=0
    )
    return loss, out


def kernel(feature, W, _trace=False, _trace_kwargs=None):
    from concourse.bass_utils import run_bass_kernel_spmd

    nc = _build()
    in_maps = _prep_inputs(feature, W)
    res = run_bass_kernel_spmd(
        nc,
        in_maps,
        list(range(NCORES)),
        trace=_trace,
        **(_trace_kwargs or {}),
    )
    loss, out = _finish(res.results)
    if _trace:
        return (loss, out), res
    return (loss, out)
